# revision 1
# baseline (speedup 1.0000x reference)
"""3-layer GCN on 8 Trainium2 NeuronCores (Bass/Tile).

Math (per layer, identical to PyG GCNConv with self-loops):
    x_{l+1} = A_hat @ (x_l @ W_l) + b_l,   A_hat = D^-1/2 (A+I) D^-1/2

We use associativity to aggregate FIRST and GEMM second:
    x_{l+1} = (A_hat @ x_l) @ W_l + b_l
so each core only runs the 128x128 GEMM on its own 1/8 of the nodes.

Sharding: node v -> core (v % 8), local row j = v // 8.  Each core owns
aggregation + GEMM for its 12500 destination rows.  Between layers an
AllGather over internal DRAM rebuilds the full (permuted) feature table
x_perm[(v%8)*12500 + v//8] = x[v] in bf16 that the next layer's gathers
read.

Per-edge work on device:
  - Edges are grouped into 128-edge chunks; each chunk's edges share one
    128-row destination tile and one 32768-row source bucket (so row
    offsets fit the SWDGE gather's int16 index format).
  - Destination tiles are grouped 4 to a "supergroup" whose aggregate
    lives in one [128, 512] PSUM bank.  Per (supergroup, bucket) a run
    of dma_gather instructions (1024 indices each -- the HW cap per
    instruction, found empirically) pulls all of its chunks' source
    rows (one 512B/256B row per edge) into SBUF, amortizing the ~1us
    SWDGE fixed overhead over 8 chunks (the baseline paid it per 128
    edges via indirect_dma_start).  Per-tile partial chunks are merged
    across tiles (the op metadata masks foreign edges), cutting padding.
  - VectorE builds S^T[e,d] = (iota[d] == dst_local[e]) * norm[e] in
    bf16, one tensor_scalar op per matmul op.
  - TensorE accumulates aggT[f,d] += msg_chunk^T @ S^T in PSUM in bf16
    (4x faster than the baseline's fp32 matmuls).  Layer 0 gathers fp32
    node_features into small rotating buffers and converts to bf16 on
    ScalarE/VectorE (alternating to balance engine load).
  - aggT feeds the GEMM directly as lhsT; bias is added with a rank-1
    (ones x bias) accumulating matmul; ScalarE copies PSUM->SBUF.

All edge metadata (int16 in-bucket gather rows in the SWDGE wrapped
layout, dst_local and norm per edge slot) is host-precomputed, padded to
128-edge chunks (pad: row=bucket base, norm=0) and streamed per
supergroup.
"""

import numpy as np

# ----------------------------------------------------------------- config

FULL_CFG = dict(
    N=100000,          # nodes
    D=128,             # feature dim (= hidden)
    CORES=8,
    TPG=4,             # dst tiles per supergroup (PSUM bank = 512 fp32)
    GSG=1,             # supergroups per gather window (msg tile size)
    BUCKET=32768,      # source rows per gather bucket (int16 idx range)
)


def _derive(cfg):
    c = dict(cfg)
    c["NPC"] = c["N"] // c["CORES"]            # nodes per core
    assert c["NPC"] * c["CORES"] == c["N"]
    c["NT"] = (c["NPC"] + 127) // 128          # dst tiles per core
    c["NSG"] = (c["NT"] + c["TPG"] - 1) // c["TPG"]
    c["NW"] = (c["NSG"] + c["GSG"] - 1) // c["GSG"]
    c["NB"] = (c["N"] + c["BUCKET"] - 1) // c["BUCKET"]
    return c


# ----------------------------------------------------- host preprocessing

def _edge_layout(cfg, dst_core, dst_local, bucket):
    """Chunk/slot layout (identical across cores, SPMD program).

    Slot order: supergroup-major, then bucket; within a bucket, each
    tile's FULL 128-edge chunks first, then one merged region packing
    the per-tile remainders (chunks there span tile boundaries; each
    (tile, merged-chunk) overlap becomes its own matmul op whose
    dst/norm metadata masks out the other tiles' edges).  One dma_gather
    per (supergroup, bucket) covers a contiguous slot range.

    Per-edge placement data is carried back via per-(tile,bucket)
    arrays: full_slot0/full_op0 (first slot/op of the full chunks),
    full128 (edges going to full chunks), rem_pos0 (position of the
    tile's remainder segment in the bucket's merged region),
    rem_slot0/rem_op0 (slot/op of the merged region's first chunk /
    the tile's first merged op).
    """
    CORES, NT, TPG, NB = cfg["CORES"], cfg["NT"], cfg["TPG"], cfg["NB"]
    tile = dst_local // 128
    key = (dst_core * NT * NB + tile * NB + bucket).astype(np.int64)
    counts = np.bincount(key, minlength=CORES * NT * NB)
    counts = counts.reshape(CORES, NT, NB)
    mx = counts.max(axis=0)                     # [NT, NB]
    full = mx // 128
    rem = mx % 128

    full_slot0 = np.zeros((NT, NB), np.int64)
    full_op0 = np.zeros((NT, NB), np.int64)
    full128 = (full * 128).astype(np.int64)
    rem_pos0 = np.zeros((NT, NB), np.int64)
    rem_slot0 = np.zeros((NT, NB), np.int64)
    rem_op0 = np.zeros((NT, NB), np.int64)

    ginfo = []
    goff = 0          # global slot offset
    ooff = 0          # global op-column offset
    for w in range(cfg["NW"]):
        sgs_idx = range(w * cfg["GSG"], min((w + 1) * cfg["GSG"], cfg["NSG"]))
        gathers = []                      # (bucket, s0_local, n_ch)
        sgs = [dict(tiles=list(range(sg * TPG, min((sg + 1) * TPG, NT))),
                    tile_ops={}) for sg in sgs_idx]
        for s in sgs:
            s["tile_ops"] = {t: [] for t in s["tiles"]}
        off = 0
        opo = 0
        for b in range(NB):
            s0 = off
            for s in sgs:
                for t in s["tiles"]:
                    f = int(full[t, b])
                    full_slot0[t, b] = goff + off
                    full_op0[t, b] = ooff + opo
                    for j in range(f):
                        s["tile_ops"][t].append((off + j, opo + j))
                    off += f
                    opo += f
                # merged remainder region for this (supergroup, bucket)
                cum = 0
                mslot0 = off
                for t in s["tiles"]:
                    r = int(rem[t, b])
                    if r == 0:
                        continue
                    rem_pos0[t, b] = cum
                    rem_slot0[t, b] = goff + mslot0
                    rem_op0[t, b] = ooff + opo
                    first, last = cum // 128, (cum + r - 1) // 128
                    for m in range(first, last + 1):
                        s["tile_ops"][t].append((mslot0 + m, opo + (m - first)))
                    opo += last - first + 1
                    cum += r
                off += -(-cum // 128)
            if off > s0:
                gathers.append((b, s0, off - s0))
        ginfo.append(dict(sgs=sgs, S_g=off, OPS_g=opo, goff=goff,
                          ops_off=ooff, gathers=gathers))
        goff += off
        ooff += opo
    return dict(ginfo=ginfo, tot_slots=goff, tot_ops=ooff, key=key,
                tile=tile, full_slot0=full_slot0, full_op0=full_op0,
                full128=full128, rem_pos0=rem_pos0, rem_slot0=rem_slot0,
                rem_op0=rem_op0)


def _fill_blobs(cfg, lay, src_rows, dst_core, dst_local, norm):
    """Per-core idx/dst/norm blobs.

    idx16: [CORES, 128, tot*8] int16 — the SWDGE gather wrapped layout:
      global gather position i (slot s = i//128, partition i%128) lives
      at [i%16 + 16k, s*8 + (i%128)//16... ] -- precisely: within a
      gather range the idx vector I[i] sits at [(i%16), i//16] of the
      range's [16, nidx/16] block, replicated 8x across partitions.
      Because every gather range starts at a slot boundary (slot = 128
      idxs = 8 columns), the global blob column for (s, p) is
      s*8 + p//16 and the partition row is p%16 (+16k replicas).
    """
    CORES = cfg["CORES"]
    tot = lay["tot_slots"]
    tot_ops = lay["tot_ops"]
    key, tile = lay["key"], lay["tile"]
    NB = cfg["NB"]
    bucket = key % NB

    order = np.argsort(key, kind="stable")
    counts_flat = np.bincount(key, minlength=CORES * cfg["NT"] * NB)
    seg_off = np.concatenate([[0], np.cumsum(counts_flat)])
    rank_sorted = np.arange(len(order)) - seg_off[key[order]]
    rank = np.empty(len(order), np.int64)
    rank[order] = rank_sorted

    f128 = lay["full128"][tile, bucket]
    is_full = rank < f128
    pos = lay["rem_pos0"][tile, bucket] + (rank - f128)   # merged position
    gslot = np.where(is_full,
                     lay["full_slot0"][tile, bucket] + rank // 128,
                     lay["rem_slot0"][tile, bucket] + pos // 128)
    part = np.where(is_full, rank % 128, pos % 128)
    opcol = np.where(is_full,
                     lay["full_op0"][tile, bucket] + rank // 128,
                     lay["rem_op0"][tile, bucket]
                     + pos // 128 - lay["rem_pos0"][tile, bucket] // 128)

    idx16 = np.zeros((CORES, 16, tot * 8), np.int16)
    meta_dst = np.zeros((CORES, 128, tot_ops), np.float32)
    meta_nrm = np.zeros((CORES, 128, tot_ops), np.float32)
    inb_row = (src_rows - bucket * cfg["BUCKET"]).astype(np.int16)
    idx16[dst_core, part % 16, gslot * 8 + part // 16] = inb_row
    meta_dst[dst_core, part, opcol] = (dst_local - tile * 128).astype(
        np.float32)
    meta_nrm[dst_core, part, opcol] = norm.astype(np.float32)
    idx128 = np.tile(idx16, (1, 8, 1))          # replicate across Q7 cores
    return idx128, meta_dst, meta_nrm


def preprocess(cfg, edge_index):
    N, CORES, NPC = cfg["N"], cfg["CORES"], cfg["NPC"]
    ei = np.asarray(edge_index).astype(np.int64)
    src = np.concatenate([ei[0], np.arange(N, dtype=np.int64)])
    dst = np.concatenate([ei[1], np.arange(N, dtype=np.int64)])
    deg = np.bincount(dst, minlength=N).astype(np.float32)
    dinv = (1.0 / np.sqrt(deg)).astype(np.float32)
    norm = dinv[src] * dinv[dst]

    dst_core = (dst % CORES).astype(np.int64)
    dst_local = (dst // CORES).astype(np.int64)

    # layer 0 gathers from node_features (original numbering);
    # layers 1,2 gather from the AllGather output (permuted numbering).
    prow = (src % CORES) * NPC + src // CORES
    b0 = src // cfg["BUCKET"]
    b12 = prow // cfg["BUCKET"]
    lay0 = _edge_layout(cfg, dst_core, dst_local, b0)
    lay12 = _edge_layout(cfg, dst_core, dst_local, b12)
    blobs0 = _fill_blobs(cfg, lay0, src, dst_core, dst_local, norm)
    blobs12 = _fill_blobs(cfg, lay12, prow, dst_core, dst_local, norm)

    def pack_meta(lay, blobs):
        idx128, meta_dst, meta_nrm = blobs
        tot = lay["tot_ops"]
        mn = np.zeros((cfg["CORES"], 128, 2 * tot), np.float32)
        for gi in lay["ginfo"]:
            o, S = gi["ops_off"], gi["OPS_g"]
            mn[:, :, 2 * o:2 * o + S] = meta_dst[:, :, o:o + S]
            mn[:, :, 2 * o + S:2 * o + 2 * S] = meta_nrm[:, :, o:o + S]
        return idx128, mn

    return lay0, lay12, pack_meta(lay0, blobs0), pack_meta(lay12, blobs12)


# -------------------------------------------------------- device program

def build_program(cfg, lay0, lay12, n_layers=3, use_collective=True,
                  est=False):
    import concourse.bass as bass  # noqa: F401
    import concourse.bacc as bacc
    import concourse.tile as tile
    import concourse.mybir as mybir

    f32 = mybir.dt.float32
    bf16 = mybir.dt.bfloat16
    i16 = mybir.dt.int16
    N, D, CORES = cfg["N"], cfg["D"], cfg["CORES"]
    NPC, TPG, NB, BUCKET = cfg["NPC"], cfg["TPG"], cfg["NB"], cfg["BUCKET"]

    nc = bacc.Bacc("TRN2", target_bir_lowering=False, debug=False,
                   num_devices=CORES)

    x0 = nc.dram_tensor("node_features", [N, D], f32, kind="ExternalInput")
    iota_in = nc.dram_tensor("iota", [128, 128], bf16, kind="ExternalInput")
    W_in = [nc.dram_tensor(f"W{l}", [D, D], f32, kind="ExternalInput")
            for l in range(3)]
    B_in = [nc.dram_tensor(f"b{l}", [1, D], f32, kind="ExternalInput")
            for l in range(3)]
    lays = {0: lay0, 12: lay12}
    idx_in = {lc: nc.dram_tensor(f"idxL{lc}", [128, lays[lc]["tot_slots"] * 8],
                                 i16, kind="ExternalInput") for lc in (0, 12)}
    mn_in = {lc: nc.dram_tensor(f"mnL{lc}", [128, 2 * lays[lc]["tot_ops"]],
                                f32, kind="ExternalInput") for lc in (0, 12)}
    y_out = nc.dram_tensor("y_out", [NPC, D], bf16 if est else f32,
                           kind="ExternalOutput")

    xs = [nc.dram_tensor(f"xslice{l}", [NPC, D], bf16) for l in range(2)]
    xg = [nc.dram_tensor(f"xgath{l}", [N, D], bf16, addr_space="Shared")
          for l in range(2)]

    with tile.TileContext(nc) as tc:
        with (
            tc.tile_pool(name="const", bufs=1) as constp,
            tc.tile_pool(name="msgf", bufs=8) as msgfp,
            tc.tile_pool(name="msgb", bufs=2) as msgbp,
            tc.tile_pool(name="meta", bufs=4) as metap,
            tc.tile_pool(name="st", bufs=16) as stp,
            tc.tile_pool(name="sb2", bufs=3) as sb2p,
            tc.tile_pool(name="psA", bufs=3, space="PSUM") as psAp,
            tc.tile_pool(name="psY", bufs=4, space="PSUM") as psYp,
        ):
            iota_sb = constp.tile([128, 128], bf16, tag="iota")
            nc.sync.dma_start(iota_sb[:], iota_in[:, :])
            ones_sb = constp.tile([1, 128], bf16, tag="ones")
            nc.vector.memset(ones_sb[:], 1.0)
            W_sb, B_sb = [], []
            for l in range(3):
                wf = constp.tile([128, 128], f32, tag=f"Wf{l}")
                nc.sync.dma_start(wf[:], W_in[l][:, :])
                w = constp.tile([128, 128], bf16, tag=f"W{l}")
                nc.scalar.copy(w[:], wf[:])
                W_sb.append(w)
                bf = constp.tile([1, 128], f32, tag=f"bf{l}")
                nc.sync.dma_start(bf[:], B_in[l][:, :])
                b = constp.tile([1, 128], bf16, tag=f"b{l}")
                nc.scalar.copy(b[:], bf[:])
                B_sb.append(b)

            cnv_i = [0]
            for l in range(n_layers):
                lc = 0 if l == 0 else 12
                lay = lays[lc]
                x_src = x0 if l == 0 else xg[l - 1]
                mdt = f32 if l == 0 else bf16
                tgt = y_out if l == n_layers - 1 else xs[l]
                ydt = bf16 if (l < n_layers - 1 or est) else f32
                for gi in lay["ginfo"]:
                    S_g = gi["S_g"]
                    OPS_g = gi["OPS_g"]
                    goff = gi["goff"]
                    ooff = gi["ops_off"]
                    msgb = msgbp.tile([128, S_g * 128], bf16, tag="msgc")
                    msgb3 = msgb[:].rearrange("p (s e) -> p s e", e=128)
                    idxt = metap.tile([128, S_g * 8], i16, tag="idx")
                    mnt = metap.tile([128, 2 * OPS_g], f32, tag="mn")
                    nc.sync.dma_start(idxt[:, :],
                                      idx_in[lc][:, goff * 8:(goff + S_g) * 8])
                    nc.sync.dma_start(mnt[:, :],
                                      mn_in[lc][:, 2 * ooff:2 * ooff
                                                 + 2 * OPS_g])
                    mna = mnt[:]
                    dstt = mna[:, :OPS_g]
                    nrmt = mna[:, OPS_g:2 * OPS_g]
                    for (b, g0, gch) in gi["gathers"]:
                        base = b * BUCKET
                        rows = min(BUCKET, N - base)
                        # HW caps one dma_gather at 1024 indices (8 chunks).
                        for s0 in range(g0, g0 + gch, 8):
                            nch = min(8, g0 + gch - s0)
                            nidx = nch * 128
                            if l == 0:
                                # fp32 source: gather into a small rotating
                                # buffer, convert to bf16 into msgb
                                msgf = msgfp.tile([128, 8 * 128], f32,
                                                  tag="msgf")
                                msgf3 = msgf[:].rearrange(
                                    "p (s e) -> p s e", e=128)
                                nc.gpsimd.dma_gather(
                                    msgf3[:, :nch, :],
                                    x_src[base:base + rows, :],
                                    idxt[:, s0 * 8:(s0 + nch) * 8],
                                    nidx, nidx, 128,
                                )
                                cnv_i[0] += 1
                                ceng = (nc.scalar.copy if cnv_i[0] % 3
                                        else nc.vector.tensor_copy)
                                ceng(msgb[:, s0 * 128:(s0 + nch) * 128],
                                     msgf[:, :nch * 128])
                            else:
                                nc.gpsimd.dma_gather(
                                    msgb3[:, s0:s0 + nch, :],
                                    x_src[base:base + rows, :],
                                    idxt[:, s0 * 8:(s0 + nch) * 8],
                                    nidx, nidx, 128,
                                )
                    for sginfo in gi["sgs"]:
                        tiles = sginfo["tiles"]
                        psA = psAp.tile([128, TPG * 128], f32, tag="psA")
                        for ti, t in enumerate(tiles):
                            ops = sginfo["tile_ops"][t]
                            for j, (s, oc) in enumerate(ops):
                                stt = stp.tile([128, 128], bf16, tag="st")
                                nc.vector.tensor_scalar(
                                    stt[:], iota_sb[:],
                                    dstt[:, oc:oc + 1],
                                    nrmt[:, oc:oc + 1],
                                    mybir.AluOpType.is_equal,
                                    mybir.AluOpType.mult,
                                )
                                nc.tensor.matmul(
                                    psA[:, ti * 128:(ti + 1) * 128],
                                    msgb3[:, s, :], stt[:],
                                    start=(j == 0),
                                    stop=(j == len(ops) - 1),
                                )
                        aggT = sb2p.tile([128, TPG * 128], bf16, tag="aggT")
                        nc.scalar.copy(aggT[:], psA[:])
                        for ti, t in enumerate(tiles):
                            psY = psYp.tile([128, 128], f32, tag="psY")
                            nc.tensor.matmul(psY[:],
                                             aggT[:, ti * 128:(ti + 1) * 128],
                                             W_sb[l][:], start=True,
                                             stop=False)
                            nc.tensor.matmul(psY[:], ones_sb[:], B_sb[l][:],
                                             start=False, stop=True)
                            ysb = sb2p.tile([128, 128], ydt, tag="ysb")
                            nc.scalar.copy(ysb[:], psY[:])
                            rows = min(128, NPC - t * 128)
                            nc.sync.dma_start(tgt[t * 128:t * 128 + rows, :],
                                              ysb[:rows, :])
                if l < n_layers - 1 and use_collective:
                    nc.gpsimd.collective_compute(
                        "AllGather",
                        mybir.AluOpType.bypass,
                        replica_groups=[list(range(CORES))],
                        ins=[xs[l][:, :].opt()],
                        outs=[xg[l][:, :].opt()],
                    )
    nc.compile()
    return nc


# ------------------------------------------------------------- execution

def make_in_maps(cfg, inputs, blobs0, blobs12):
    import ml_dtypes

    CORES, D = cfg["CORES"], cfg["D"]
    idx0, mn0 = blobs0
    idx12, mn12 = blobs12
    iota = np.tile(np.arange(128, dtype=ml_dtypes.bfloat16), (128, 1))
    nf = np.ascontiguousarray(np.asarray(inputs["node_features"],
                                         dtype=np.float32))
    in_maps = []
    for c in range(CORES):
        m = {
            "node_features": nf,
            "iota": iota,
            "idxL0": np.ascontiguousarray(idx0[c]),
            "mnL0": np.ascontiguousarray(mn0[c]),
            "idxL12": np.ascontiguousarray(idx12[c]),
            "mnL12": np.ascontiguousarray(mn12[c]),
        }
        for l in range(3):
            m[f"W{l}"] = np.ascontiguousarray(
                np.asarray(inputs[f"W{l}"], dtype=np.float32))
            m[f"b{l}"] = np.ascontiguousarray(
                np.asarray(inputs[f"b{l}"], dtype=np.float32).reshape(1, D))
        in_maps.append(m)
    return in_maps


def unshard_output(cfg, results):
    N, D, CORES, NPC = cfg["N"], cfg["D"], cfg["CORES"], cfg["NPC"]
    out = np.empty((N, D), np.float32)
    for c in range(CORES):
        out[c::CORES] = results[c]["y_out"][:NPC]
    return out


_CACHE = {}


def kernel(**inputs) -> np.ndarray:
    import time

    cfg = _derive(FULL_CFG)
    ekey = hash(np.asarray(inputs["edge_index"]).tobytes())
    if ekey in _CACHE:
        blobs0, blobs12, nc = _CACHE[ekey]
    else:
        lay0, lay12, blobs0, blobs12 = preprocess(cfg, inputs["edge_index"])
        nc = build_program(cfg, lay0, lay12)
        _CACHE.clear()
        _CACHE[ekey] = (blobs0, blobs12, nc)
    in_maps = make_in_maps(cfg, inputs, blobs0, blobs12)
    from concourse import bass_utils

    # The axon-tunneled device occasionally dies mid-run
    # (NRT_EXEC_UNIT_UNRECOVERABLE) and the worker restarts itself over
    # the next minute or two; retry a few times before giving up.
    last_exc = None
    for attempt, backoff_s in enumerate([0, 90, 180, 240]):
        if backoff_s:
            time.sleep(backoff_s)
        try:
            res = bass_utils.run_bass_kernel_spmd(
                nc, in_maps, core_ids=list(range(cfg["CORES"])))
            return unshard_output(cfg, res.results)
        except Exception as exc:  # transient worker/device failures
            last_exc = exc
            try:
                import jax
                jax.clear_caches()
            except Exception:
                pass
    raise last_exc

